# revision 31
# baseline (speedup 1.0000x reference)
"""AttentionPool (segment softmax + weighted segment sum) on 8 TRN2 cores.

kernel(x, batch, W1, b1, W2, b2) -> pooled [4096, 256] f32

Strategy (data-parallel over graphs, per the sharding hint):
  - batch is sorted, so nodes of each graph are contiguous. Each core gets
    512 consecutive graphs = 4 windows of GW=128 graphs. Each window's nodes
    are padded to a common NW rows (multiple of 512).
  - The host prepares x in BOTH layouts the PE needs, as fp16 (same total
    HBM bytes as one fp32 copy, and fp16 matmuls run 4x faster than fp32):
      xn: node-on-partition blocks [128, KCH, C+1] with a baked ones column
          (feeds the one-hot segment-accumulate matmul; the ones column
          accumulates the softmax denominators)
      xt: channel-on-partition blocks [128, 2, BLK] (feeds the MLP matmul;
          replaces the baseline's on-device PE transposes)
  - Per 512-node block: h = W1^T @ xt (2 fp16 matmuls, K=128 each), tanh on
    ACT (+b1) -> th fp16, per-chunk score matmuls with W2 (scores land
    node-on-partition), exp on ACT, one-hot build on DVE (iota==gl)*st, and
    4 accumulating fp16 matmuls into a per-window [GW, C+1] PSUM bank.
    b2 is skipped (softmax invariant); max-subtraction is skipped
    (|score| <= sum|W2| ~ 10, exp fits fp32/fp16 comfortably).
  - 3-stage software pipeline (emit h(i), scores(i-1), accumulate(i-2)) so
    the in-order PE stream never waits on ACT/DVE results of the same block.
  - Finalize per window: pooled = acc_x / (acc_denom + 1e-16), DMA out.

Padding rows carry local-graph-id -1 so their one-hot row is all zero: they
contribute to neither numerator nor denominator.
"""

import os
from contextlib import ExitStack

import numpy as np
from ml_dtypes import bfloat16  # noqa: F401  (available if needed)

import concourse.bass as bass
import concourse.mybir as mybir
import concourse.tile as tile
from concourse import bacc, bass_utils

FP32 = mybir.dt.float32
F16 = mybir.dt.float16
F8 = mybir.dt.float8e4  # e4m3: score-path x only (softmax tolerates ~1e-2)
U8 = mybir.dt.uint8
NP16 = np.float16
NP8 = mybir.dt.np(F8)
XN_B = KCH * (C + 1) * 2  # 2056 B/partition: node-major fp16 + ones col
XT_B = 2 * BLK  # 1024 B/partition: channel-major fp8
PB = XN_B + XT_B  # packed bytes per block per partition
C = 256
BLK = 512
KCH = BLK // 128  # 4 chunks of 128 nodes per block
N_CORES = 8
GW = 128  # graphs per window
NWIN = 4  # windows per core -> 512 graphs per core

_NC_CACHE = {}


def _build_nc(nwin, nw, gw, repeat=1):
    # repeat > 1 re-runs the whole computation (same inputs/outputs) inside
    # one NEFF; used only for overhead-cancelling timing measurements.
    assert nw % BLK == 0 and gw <= 128
    bpw = nw // BLK
    nblk = nwin * bpw

    nc = bacc.Bacc(None, target_bir_lowering=False)

    assert nblk % 2 == 0
    # both x layouts byte-packed, two blocks per DMA (fewer, larger transfers)
    xc = nc.dram_tensor("xc", [nblk // 2, 128, 2, PB], U8, kind="ExternalInput")
    gl = nc.dram_tensor("gl", [nwin, 128, bpw * KCH], FP32, kind="ExternalInput")
    w1 = nc.dram_tensor("w1", [128, 2, 128], F16, kind="ExternalInput")
    b1v = nc.dram_tensor("b1v", [128, 1], FP32, kind="ExternalInput")
    w2 = nc.dram_tensor("w2", [128, 1], F16, kind="ExternalInput")
    pooled = nc.dram_tensor("pooled", [nwin * gw, C], FP32, kind="ExternalOutput")

    with tile.TileContext(nc) as tc, ExitStack() as ctx:
        consts = ctx.enter_context(tc.tile_pool(name="consts", bufs=1))
        glp = ctx.enter_context(tc.tile_pool(name="glp", bufs=2))
        xcp = ctx.enter_context(tc.tile_pool(name="xcp", bufs=5))
        thp = ctx.enter_context(tc.tile_pool(name="thp", bufs=3))
        sp = ctx.enter_context(tc.tile_pool(name="sp", bufs=3))
        oep = ctx.enter_context(tc.tile_pool(name="oep", bufs=3))
        outp = ctx.enter_context(tc.tile_pool(name="outp", bufs=2))
        ps_h = ctx.enter_context(tc.tile_pool(name="ps_h", bufs=2, space="PSUM"))
        ps_s = ctx.enter_context(tc.tile_pool(name="ps_s", bufs=2, space="PSUM"))
        ps_acc = ctx.enter_context(tc.tile_pool(name="ps_acc", bufs=2, space="PSUM"))

        w1_sb = consts.tile([128, 2, 128], F16)
        nc.sync.dma_start(out=w1_sb[:], in_=w1[:])
        b1_sb = consts.tile([128, 1], FP32)
        nc.sync.dma_start(out=b1_sb[:], in_=b1v[:])
        w2_sb = consts.tile([128, 1], F16)
        nc.sync.dma_start(out=w2_sb[:], in_=w2[:])
        iota_i = consts.tile([128, gw], mybir.dt.int32)
        nc.gpsimd.iota(iota_i[:], pattern=[[1, gw]], base=0, channel_multiplier=0)
        iota_g = consts.tile([128, gw], FP32)
        nc.vector.tensor_copy(out=iota_g[:], in_=iota_i[:])


        state = {}
        blocks = [(w, b) for w in range(repeat * nwin) for b in range(bpw)]

        def stage_a(i):
            # DMA loads + MLP hidden matmul + tanh
            w, b = blocks[i]
            if b == 0:
                glt = glp.tile([128, bpw * KCH], FP32, name="glt", tag="gl")
                state[("gl", w)] = glt
                nc.sync.dma_start(out=glt[:], in_=gl[w % nwin])
            gblk = (w % nwin) * bpw + b
            if gblk % 2 == 0:
                xcb = xcp.tile([128, 2, PB], U8, name="xcb", tag="xc")
                nc.sync.dma_start(out=xcb[:], in_=xc[gblk // 2])
                state["xcpair"] = xcb
            xcb = state["xcpair"]
            j = gblk % 2
            xnb = xcb[:, j, :XN_B].bitcast(F16).rearrange("p (k c) -> p k c", k=KCH)
            xtb = xcb[:, j, XN_B:PB].bitcast(F8).rearrange("p (cb n) -> p cb n", cb=2)

            h_ps = ps_h.tile([128, BLK], FP32, tag="h")
            for cb in range(2):
                nc.tensor.matmul(
                    out=h_ps[:],
                    lhsT=w1_sb[:, cb, :],
                    rhs=xtb[:, cb, :],
                    start=(cb == 0),
                    stop=(cb == 1),
                )
            th = thp.tile([128, BLK], F16, tag="th")
            nc.scalar.activation(
                out=th[:],
                in_=h_ps[:],
                func=mybir.ActivationFunctionType.Tanh,
                bias=b1_sb[:],
                scale=1.0,
            )
            return (w, b, xnb, th)

        def stage_c(args):
            # score matmuls + exp + one-hot build
            w, b, xnb, th = args
            st_ps = ps_s.tile([128, KCH], FP32, tag="stps")
            for k in range(KCH):
                nc.tensor.matmul(
                    out=st_ps[:, k : k + 1],
                    lhsT=th[:, 128 * k : 128 * (k + 1)],
                    rhs=w2_sb[:],
                )
            st = sp.tile([128, KCH], FP32, tag="st")
            nc.scalar.activation(
                out=st[:], in_=st_ps[:], func=mybir.ActivationFunctionType.Exp
            )
            glt = state[("gl", w)]
            oe = oep.tile([128, KCH, gw], F16, tag="oe")
            for k in range(KCH):
                nc.vector.tensor_scalar(
                    out=oe[:, k, :],
                    in0=iota_g[:],
                    scalar1=glt[:, KCH * b + k : KCH * b + k + 1],
                    scalar2=st[:, k : k + 1],
                    op0=mybir.AluOpType.is_equal,
                    op1=mybir.AluOpType.mult,
                )
            return (w, b, xnb, oe)

        def stage_e(args):
            # segment accumulate (+ window finalize)
            w, b, xnb, oe = args
            first = b == 0
            last = b == bpw - 1
            if first:
                state[("acc", w)] = ps_acc.tile(
                    [gw, C + 1], FP32, name="acc", tag="acc"
                )
            acc = state[("acc", w)]
            for k in range(KCH):
                nc.tensor.matmul(
                    out=acc[:],
                    lhsT=oe[:, k, :],
                    rhs=xnb[:, k, :],
                    start=(first and k == 0),
                    stop=(last and k == KCH - 1),
                )
            if last:
                recip = outp.tile([gw, 1], FP32, tag="recip")
                nc.vector.tensor_scalar_add(recip[:], acc[:, C : C + 1], 1e-16)
                nc.vector.reciprocal(out=recip[:], in_=recip[:])
                out_sb = outp.tile([gw, C], FP32, tag="out_sb")
                nc.vector.tensor_scalar_mul(out_sb[:], acc[:, :C], recip[:])
                nc.sync.dma_start(
                    out=pooled[(w % nwin) * gw : (w % nwin + 1) * gw, :],
                    in_=out_sb[:],
                )
                del state[("acc", w)], state[("gl", w)]

        pa, pc = [], []
        for i in range(len(blocks)):
            pa.append(stage_a(i))
            if len(pa) > 1:
                pc.append(stage_c(pa.pop(0)))
            if len(pc) > 1:
                stage_e(pc.pop(0))
        while pa:
            pc.append(stage_c(pa.pop(0)))
        while pc:
            stage_e(pc.pop(0))

    nc.compile()
    return nc


def _shard_inputs(x, batch, W1, b1, W2, nw):
    n_graphs = N_CORES * NWIN * GW
    bpw = nw // BLK
    kj = bpw * KCH
    x32 = np.ascontiguousarray(x, dtype=np.float32)
    x16 = x32.astype(NP16)
    x8 = x32.astype(NP8)
    batch = np.asarray(batch)

    wstarts = np.searchsorted(batch, np.arange(0, n_graphs + 1, GW))
    W1 = np.asarray(W1, dtype=np.float32)
    w1_host = np.ascontiguousarray(
        W1.reshape(2, 128, 128).transpose(1, 0, 2)
    ).astype(NP16)
    b1_host = np.asarray(b1, dtype=np.float32).reshape(128, 1)
    w2_host = np.asarray(W2, dtype=np.float32).reshape(128, 1).astype(NP16)

    in_maps = []
    for c in range(N_CORES):
        xn = np.zeros((NWIN * bpw, 128, KCH, C + 1), dtype=NP16)
        xn[..., C] = 1.0
        xt = np.zeros((NWIN * bpw, 128, 2, BLK), dtype=NP8)
        gl = np.full((NWIN, 128, kj), -1.0, dtype=np.float32)
        for wl in range(NWIN):
            wg = c * NWIN + wl
            lo, hi = int(wstarts[wg]), int(wstarts[wg + 1])
            cnt = hi - lo
            assert cnt <= nw, f"window {wg} has {cnt} nodes > NW={nw}"
            xpad = np.zeros((nw, C), dtype=NP16)
            xpad[:cnt] = x16[lo:hi]
            # node n = b*512 + k*128 + p  ->  xn[b, p, k, :C]
            xn[wl * bpw : (wl + 1) * bpw, :, :, :C] = xpad.reshape(
                bpw, KCH, 128, C
            ).transpose(0, 2, 1, 3)
            xpad8 = np.zeros((nw, C), dtype=NP8)
            xpad8[:cnt] = x8[lo:hi]
            # channel ch = cb*128 + p    ->  xt[b, p, cb, n512]
            xt[wl * bpw : (wl + 1) * bpw] = xpad8.reshape(
                bpw, BLK, 2, 128
            ).transpose(0, 3, 2, 1)
            glpad = np.full((nw,), -1.0, dtype=np.float32)
            glpad[:cnt] = (batch[lo:hi] - wg * GW).astype(np.float32)
            gl[wl] = glpad.reshape(bpw, KCH, 128).transpose(2, 0, 1).reshape(128, kj)
        nblk = NWIN * bpw
        xcat = np.concatenate(
            [
                xn.reshape(nblk, 128, XN_B // 2).view(np.uint8),
                xt.reshape(nblk, 128, XT_B).view(np.uint8),
            ],
            axis=2,
        )
        xc = np.ascontiguousarray(
            xcat.reshape(nblk // 2, 2, 128, PB).transpose(0, 2, 1, 3)
        )
        in_maps.append(
            {
                "xc": xc,
                "gl": gl,
                "w1": w1_host,
                "b1v": b1_host,
                "w2": w2_host,
            }
        )
    return in_maps


def kernel(x, batch, W1, b1, W2, b2):
    x = np.asarray(x)
    batch = np.asarray(batch)
    n_graphs = N_CORES * NWIN * GW
    assert x.shape[1] == C and batch.shape[0] == x.shape[0]

    # padded nodes per window, from the actual data
    wstarts = np.searchsorted(batch, np.arange(0, n_graphs + 1, GW))
    max_win = int(np.diff(wstarts).max())
    nw = max(BLK, -(-max_win // BLK) * BLK)

    key = (NWIN, nw, GW)
    if key not in _NC_CACHE:
        _NC_CACHE[key] = _build_nc(*key)
    nc = _NC_CACHE[key]

    in_maps = _shard_inputs(x, batch, W1, b1, W2, nw)
    trace = os.environ.get("ATTN_TRACE") == "1"
    res = bass_utils.run_bass_kernel_spmd(
        nc,
        in_maps,
        core_ids=list(range(N_CORES)),
        trace=trace,
        **(
            {"trace_cores": [0], "trace_kwargs": {"title": "attnpool"}}
            if trace
            else {}
        ),
    )
    if trace:
        kernel.last_results = res
    out = np.concatenate(
        [res.results[c]["pooled"] for c in range(N_CORES)], axis=0
    ).astype(np.float32)
    return out


# revision 32
# speedup vs baseline: 1.1411x; 1.1411x over previous
"""AttentionPool (segment softmax + weighted segment sum) on 8 TRN2 cores.

kernel(x, batch, W1, b1, W2, b2) -> pooled [4096, 256] f32

Strategy (data-parallel over graphs, per the sharding hint):
  - batch is sorted, so nodes of each graph are contiguous. Each core gets
    512 consecutive graphs = 4 windows of GW=128 graphs. Each window's nodes
    are padded to a common NW rows (multiple of 512).
  - The host prepares x in BOTH layouts the PE needs, as fp16 (same total
    HBM bytes as one fp32 copy, and fp16 matmuls run 4x faster than fp32):
      xn: node-on-partition blocks [128, KCH, C+1] with a baked ones column
          (feeds the one-hot segment-accumulate matmul; the ones column
          accumulates the softmax denominators)
      xt: channel-on-partition blocks [128, 2, BLK] (feeds the MLP matmul;
          replaces the baseline's on-device PE transposes)
  - Per 512-node block: h = W1^T @ xt (2 fp16 matmuls, K=128 each), tanh on
    ACT (+b1) -> th fp16, per-chunk score matmuls with W2 (scores land
    node-on-partition), exp on ACT, one-hot build on DVE (iota==gl)*st, and
    4 accumulating fp16 matmuls into a per-window [GW, C+1] PSUM bank.
    b2 is skipped (softmax invariant); max-subtraction is skipped
    (|score| <= sum|W2| ~ 10, exp fits fp32/fp16 comfortably).
  - 3-stage software pipeline (emit h(i), scores(i-1), accumulate(i-2)) so
    the in-order PE stream never waits on ACT/DVE results of the same block.
  - Finalize per window: pooled = acc_x / (acc_denom + 1e-16), DMA out.

Padding rows carry local-graph-id -1 so their one-hot row is all zero: they
contribute to neither numerator nor denominator.
"""

import os
from contextlib import ExitStack

import numpy as np
from ml_dtypes import bfloat16  # noqa: F401  (available if needed)

import concourse.bass as bass
import concourse.mybir as mybir
import concourse.tile as tile
from concourse import bacc, bass_utils

FP32 = mybir.dt.float32
F16 = mybir.dt.float16
F8 = mybir.dt.float8e4  # e4m3: score-path x only (softmax tolerates ~1e-2)
U8 = mybir.dt.uint8
NP16 = np.float16
NP8 = mybir.dt.np(F8)
XN_B = KCH * (C + 1) * 2  # 2056 B/partition: node-major fp16 + ones col
XT_B = 2 * BLK  # 1024 B/partition: channel-major fp8
PB = XN_B + XT_B  # packed bytes per block per partition
QB = 4  # blocks per DMA transfer (QB*PB*128 = 1.58 MB per transfer)
C = 256
BLK = 512
KCH = BLK // 128  # 4 chunks of 128 nodes per block
N_CORES = 8
GW = 128  # graphs per window
NWIN = 4  # windows per core -> 512 graphs per core

_NC_CACHE = {}


def _build_nc(nwin, nw, gw, repeat=1):
    # repeat > 1 re-runs the whole computation (same inputs/outputs) inside
    # one NEFF; used only for overhead-cancelling timing measurements.
    assert nw % BLK == 0 and gw <= 128
    bpw = nw // BLK
    nblk = nwin * bpw

    nc = bacc.Bacc(None, target_bir_lowering=False)

    assert nblk % QB == 0
    # both x layouts byte-packed, QB blocks per DMA (fewer, larger transfers)
    xc = nc.dram_tensor("xc", [nblk // QB, 128, QB, PB], U8, kind="ExternalInput")
    gl = nc.dram_tensor("gl", [nwin, 128, bpw * KCH], FP32, kind="ExternalInput")
    w1 = nc.dram_tensor("w1", [128, 2, 128], F16, kind="ExternalInput")
    b1v = nc.dram_tensor("b1v", [128, 1], FP32, kind="ExternalInput")
    w2 = nc.dram_tensor("w2", [128, 1], F16, kind="ExternalInput")
    pooled = nc.dram_tensor("pooled", [nwin * gw, C], FP32, kind="ExternalOutput")

    with tile.TileContext(nc) as tc, ExitStack() as ctx:
        consts = ctx.enter_context(tc.tile_pool(name="consts", bufs=1))
        glp = ctx.enter_context(tc.tile_pool(name="glp", bufs=2))
        xcp = ctx.enter_context(tc.tile_pool(name="xcp", bufs=5))
        thp = ctx.enter_context(tc.tile_pool(name="thp", bufs=3))
        sp = ctx.enter_context(tc.tile_pool(name="sp", bufs=3))
        oep = ctx.enter_context(tc.tile_pool(name="oep", bufs=3))
        outp = ctx.enter_context(tc.tile_pool(name="outp", bufs=2))
        ps_h = ctx.enter_context(tc.tile_pool(name="ps_h", bufs=2, space="PSUM"))
        ps_s = ctx.enter_context(tc.tile_pool(name="ps_s", bufs=2, space="PSUM"))
        ps_acc = ctx.enter_context(tc.tile_pool(name="ps_acc", bufs=2, space="PSUM"))

        w1_sb = consts.tile([128, 2, 128], F16)
        nc.sync.dma_start(out=w1_sb[:], in_=w1[:])
        b1_sb = consts.tile([128, 1], FP32)
        nc.sync.dma_start(out=b1_sb[:], in_=b1v[:])
        w2_sb = consts.tile([128, 1], F16)
        nc.sync.dma_start(out=w2_sb[:], in_=w2[:])
        iota_i = consts.tile([128, gw], mybir.dt.int32)
        nc.gpsimd.iota(iota_i[:], pattern=[[1, gw]], base=0, channel_multiplier=0)
        iota_g = consts.tile([128, gw], FP32)
        nc.vector.tensor_copy(out=iota_g[:], in_=iota_i[:])


        state = {}
        blocks = [(w, b) for w in range(repeat * nwin) for b in range(bpw)]

        def stage_a(i):
            # DMA loads + MLP hidden matmul + tanh
            w, b = blocks[i]
            if b == 0:
                glt = glp.tile([128, bpw * KCH], FP32, name="glt", tag="gl")
                state[("gl", w)] = glt
                nc.sync.dma_start(out=glt[:], in_=gl[w % nwin])
            gblk = (w % nwin) * bpw + b
            if gblk % QB == 0:
                xcb = xcp.tile([128, QB, PB], U8, name="xcb", tag="xc")
                nc.sync.dma_start(out=xcb[:], in_=xc[gblk // QB])
                state["xcpair"] = xcb
            xcb = state["xcpair"]
            j = gblk % QB
            xnb = xcb[:, j, :XN_B].bitcast(F16).rearrange("p (k c) -> p k c", k=KCH)
            xtb = xcb[:, j, XN_B:PB].bitcast(F8).rearrange("p (cb n) -> p cb n", cb=2)

            h_ps = ps_h.tile([128, BLK], FP32, tag="h")
            for cb in range(2):
                nc.tensor.matmul(
                    out=h_ps[:],
                    lhsT=w1_sb[:, cb, :],
                    rhs=xtb[:, cb, :],
                    start=(cb == 0),
                    stop=(cb == 1),
                )
            th = thp.tile([128, BLK], F16, tag="th")
            nc.scalar.activation(
                out=th[:],
                in_=h_ps[:],
                func=mybir.ActivationFunctionType.Tanh,
                bias=b1_sb[:],
                scale=1.0,
            )
            return (w, b, xnb, th)

        def stage_c(args):
            # score matmuls + exp + one-hot build
            w, b, xnb, th = args
            st_ps = ps_s.tile([128, KCH], FP32, tag="stps")
            for k in range(KCH):
                nc.tensor.matmul(
                    out=st_ps[:, k : k + 1],
                    lhsT=th[:, 128 * k : 128 * (k + 1)],
                    rhs=w2_sb[:],
                )
            st = sp.tile([128, KCH], FP32, tag="st")
            nc.scalar.activation(
                out=st[:], in_=st_ps[:], func=mybir.ActivationFunctionType.Exp
            )
            glt = state[("gl", w)]
            oe = oep.tile([128, KCH, gw], F16, tag="oe")
            for k in range(KCH):
                nc.vector.tensor_scalar(
                    out=oe[:, k, :],
                    in0=iota_g[:],
                    scalar1=glt[:, KCH * b + k : KCH * b + k + 1],
                    scalar2=st[:, k : k + 1],
                    op0=mybir.AluOpType.is_equal,
                    op1=mybir.AluOpType.mult,
                )
            return (w, b, xnb, oe)

        def stage_e(args):
            # segment accumulate (+ window finalize)
            w, b, xnb, oe = args
            first = b == 0
            last = b == bpw - 1
            if first:
                state[("acc", w)] = ps_acc.tile(
                    [gw, C + 1], FP32, name="acc", tag="acc"
                )
            acc = state[("acc", w)]
            for k in range(KCH):
                nc.tensor.matmul(
                    out=acc[:],
                    lhsT=oe[:, k, :],
                    rhs=xnb[:, k, :],
                    start=(first and k == 0),
                    stop=(last and k == KCH - 1),
                )
            if last:
                recip = outp.tile([gw, 1], FP32, tag="recip")
                nc.vector.tensor_scalar_add(recip[:], acc[:, C : C + 1], 1e-16)
                nc.vector.reciprocal(out=recip[:], in_=recip[:])
                out_sb = outp.tile([gw, C], FP32, tag="out_sb")
                nc.vector.tensor_scalar_mul(out_sb[:], acc[:, :C], recip[:])
                nc.sync.dma_start(
                    out=pooled[(w % nwin) * gw : (w % nwin + 1) * gw, :],
                    in_=out_sb[:],
                )
                del state[("acc", w)], state[("gl", w)]

        pa, pc = [], []
        for i in range(len(blocks)):
            pa.append(stage_a(i))
            if len(pa) > 1:
                pc.append(stage_c(pa.pop(0)))
            if len(pc) > 1:
                stage_e(pc.pop(0))
        while pa:
            pc.append(stage_c(pa.pop(0)))
        while pc:
            stage_e(pc.pop(0))

    nc.compile()
    return nc


def _shard_inputs(x, batch, W1, b1, W2, nw):
    n_graphs = N_CORES * NWIN * GW
    bpw = nw // BLK
    kj = bpw * KCH
    x32 = np.ascontiguousarray(x, dtype=np.float32)
    x16 = x32.astype(NP16)
    x8 = x32.astype(NP8)
    batch = np.asarray(batch)

    wstarts = np.searchsorted(batch, np.arange(0, n_graphs + 1, GW))
    W1 = np.asarray(W1, dtype=np.float32)
    w1_host = np.ascontiguousarray(
        W1.reshape(2, 128, 128).transpose(1, 0, 2)
    ).astype(NP16)
    b1_host = np.asarray(b1, dtype=np.float32).reshape(128, 1)
    w2_host = np.asarray(W2, dtype=np.float32).reshape(128, 1).astype(NP16)

    in_maps = []
    for c in range(N_CORES):
        xn = np.zeros((NWIN * bpw, 128, KCH, C + 1), dtype=NP16)
        xn[..., C] = 1.0
        xt = np.zeros((NWIN * bpw, 128, 2, BLK), dtype=NP8)
        gl = np.full((NWIN, 128, kj), -1.0, dtype=np.float32)
        for wl in range(NWIN):
            wg = c * NWIN + wl
            lo, hi = int(wstarts[wg]), int(wstarts[wg + 1])
            cnt = hi - lo
            assert cnt <= nw, f"window {wg} has {cnt} nodes > NW={nw}"
            xpad = np.zeros((nw, C), dtype=NP16)
            xpad[:cnt] = x16[lo:hi]
            # node n = b*512 + k*128 + p  ->  xn[b, p, k, :C]
            xn[wl * bpw : (wl + 1) * bpw, :, :, :C] = xpad.reshape(
                bpw, KCH, 128, C
            ).transpose(0, 2, 1, 3)
            xpad8 = np.zeros((nw, C), dtype=NP8)
            xpad8[:cnt] = x8[lo:hi]
            # channel ch = cb*128 + p    ->  xt[b, p, cb, n512]
            xt[wl * bpw : (wl + 1) * bpw] = xpad8.reshape(
                bpw, BLK, 2, 128
            ).transpose(0, 3, 2, 1)
            glpad = np.full((nw,), -1.0, dtype=np.float32)
            glpad[:cnt] = (batch[lo:hi] - wg * GW).astype(np.float32)
            gl[wl] = glpad.reshape(bpw, KCH, 128).transpose(2, 0, 1).reshape(128, kj)
        nblk = NWIN * bpw
        xcat = np.concatenate(
            [
                xn.reshape(nblk, 128, XN_B // 2).view(np.uint8),
                xt.reshape(nblk, 128, XT_B).view(np.uint8),
            ],
            axis=2,
        )
        xc = np.ascontiguousarray(
            xcat.reshape(nblk // QB, QB, 128, PB).transpose(0, 2, 1, 3)
        )
        in_maps.append(
            {
                "xc": xc,
                "gl": gl,
                "w1": w1_host,
                "b1v": b1_host,
                "w2": w2_host,
            }
        )
    return in_maps


def kernel(x, batch, W1, b1, W2, b2):
    x = np.asarray(x)
    batch = np.asarray(batch)
    n_graphs = N_CORES * NWIN * GW
    assert x.shape[1] == C and batch.shape[0] == x.shape[0]

    # padded nodes per window, from the actual data
    wstarts = np.searchsorted(batch, np.arange(0, n_graphs + 1, GW))
    max_win = int(np.diff(wstarts).max())
    nw = max(BLK, -(-max_win // BLK) * BLK)

    key = (NWIN, nw, GW)
    if key not in _NC_CACHE:
        _NC_CACHE[key] = _build_nc(*key)
    nc = _NC_CACHE[key]

    in_maps = _shard_inputs(x, batch, W1, b1, W2, nw)
    trace = os.environ.get("ATTN_TRACE") == "1"
    res = bass_utils.run_bass_kernel_spmd(
        nc,
        in_maps,
        core_ids=list(range(N_CORES)),
        trace=trace,
        **(
            {"trace_cores": [0], "trace_kwargs": {"title": "attnpool"}}
            if trace
            else {}
        ),
    )
    if trace:
        kernel.last_results = res
    out = np.concatenate(
        [res.results[c]["pooled"] for c in range(N_CORES)], axis=0
    ).astype(np.float32)
    return out
